# revision 36
# baseline (speedup 1.0000x reference)
# Distributed sparse-attention kernel for Trainium2 (8 NeuronCores).
#
# Sharding: core c = (batch b = c//2, head-group g = c%2 of 8 heads).
# Each core computes, for its (b, g):
#   q  = meancenter(x) @ Wc          (LN rstd cancels under l2norm; gamma and
#                                     mean-centering folded into Wc on host)
#   kv = [prefix; x] @ Wkv.T         (MQA single head, replicated per batch)
#   qn = l2norm(q) ; kn = l2norm(k) * (8 * q_scale * k_scale)
#   logits[c_key, i_query] = kn.T qn  (computed transposed, only the causal
#                                     triangle over x-cols and a 16-wide band
#                                     over prefix-cols)
#   P = exp(logits) * exp_bias       (bias/causal/key-mask folded into a
#                                     multiplicative exp(bias) table on host)
#   avT[d, i] = sum_j P[j, i] v[j, d], denom via an appended ones column of v
#   outT_partial = Wo_g.T @ (avT / denom)
# Host sums the two head-group partials per batch and transposes back.

import numpy as np

B, N, P, DIM, HEADS, DH = 4, 1024, 1024, 1024, 16, 64
HL = 8                 # heads per core
FL = HL * DH           # 512 local q features
J = P + N              # 2048 keys
WIND = 16              # prefix cond-window
BW = 144               # band tile width (128 cols + 16 window - 1, padded)
NEG = -1e30
CORES = list(range(8))


def _build_band_mask():
    # band[r, t] = 1 if key col (c0+r) is attended by query (c0+t):  0 <= t-r < WIND
    r = np.arange(128)[:, None]
    t = np.arange(BW)[None, :]
    return ((t - r >= 0) & (t - r < WIND)).astype(np.float32)


def _patch_tile_drain():
    """walrus in this image only encodes ~2 sem waits on a CTRL (Drain/Nop)
    instruction; Tile's exit drain attaches every outstanding sem wait to a
    single drain.  Split the waits across extra sync-engine nops."""
    import concourse.tile as tile_mod
    from concourse import mybir
    from concourse.vector_clock import ScopedClock

    if getattr(tile_mod.TileContext, "_drain_split_patch", False):
        return
    MAXW = 1

    _ENGS = {
        mybir.EngineType.PE, mybir.EngineType.Activation,
        mybir.EngineType.Pool, mybir.EngineType.DVE, mybir.EngineType.SP,
    }
    _LIMITS = {}
    _nsplit = [0]
    orig_add = tile_mod.TileContext._add_instruction

    def _add_instruction(self, inst):
        si = inst.sync_info
        lim = _LIMITS.get(inst.engine, 1)
        if (si is not None and si.on_wait and len(si.on_wait) > lim
                and inst.engine in _ENGS):
            waits = list(si.on_wait)
            keep = waits[:lim]
            rest = waits[lim:]
            inst.sync_info = mybir.SyncInfo(
                on_wait=keep, on_update=list(si.on_update or []))
            for i in range(0, len(rest), MAXW):
                _nsplit[0] += 1
                nop = mybir.InstNoOp(
                    name=f"{inst.name}-ws{_nsplit[0]}", ins=[], outs=[])
                nop.engine = inst.engine
                nop.sync_info = mybir.SyncInfo(
                    on_wait=rest[i:i + MAXW], on_update=[])
                orig_add(self, nop)
        orig_add(self, inst)

    tile_mod.TileContext._add_instruction = _add_instruction

    def _drain_and_barrier(self, tick_clock, wait_clock):
        drain_inst = self.nc.sync.drain()
        wait_clock.add_sem_waits(
            drain_inst.ins, ScopedClock({None: tick_clock.global_clock})
        )
        si = drain_inst.ins.sync_info
        waits = list(si.on_wait or []) if si is not None else []
        if len(waits) > MAXW:
            ups = list(si.on_update or []) if si is not None else []
            drain_inst.ins.sync_info = mybir.SyncInfo(on_wait=[], on_update=ups)
            for i in range(0, len(waits), MAXW):
                nop = self.nc.sync.nop(nofuse=True)
                nop.ins.sync_info = mybir.SyncInfo(
                    on_wait=waits[i:i + MAXW], on_update=[])
        self.nc.all_engine_barrier()
        assert self.sems is not None
        popped = self.nc._tile_sem_poison_stack.pop()
        assert popped is self._sem_poison
        self.nc.clear_and_free_semaphores(list(self.sems.allocated().values()))
        self.nc.all_engine_barrier()

    tile_mod.TileContext._drain_and_barrier = _drain_and_barrier
    tile_mod.TileContext._drain_split_patch = True


def _build_nc():
    import ml_dtypes
    import concourse.bass as bass
    import concourse.tile as tile
    from concourse import mybir

    _patch_tile_drain()

    f32 = mybir.dt.float32
    bf16 = mybir.dt.bfloat16
    bf = ml_dtypes.bfloat16

    nc = bass.Bass("TRN2", target_bir_lowering=False, debug=False)

    xT = nc.dram_tensor("xT", [DIM, N], bf16, kind="ExternalInput").ap()
    ctxT = nc.dram_tensor("ctxT", [DIM, P], bf16, kind="ExternalInput").ap()
    biasT = nc.dram_tensor("biasT", [HL, N, N], bf16, kind="ExternalInput").ap()
    wc = nc.dram_tensor("wc", [DIM, FL], bf16, kind="ExternalInput").ap()
    wkv = nc.dram_tensor("wkv", [DIM, 2 * DH], bf16, kind="ExternalInput").ap()
    wo = nc.dram_tensor("wo", [FL, DIM], bf16, kind="ExternalInput").ap()
    sdk = nc.dram_tensor("sdk", [DH, 1], f32, kind="ExternalInput").ap()
    outT = nc.dram_tensor("outT", [DIM, N], f32, kind="ExternalOutput").ap()

    bandm_dram = nc.inline_tensor(_build_band_mask().astype(bf), "bandm").ap()
    idup_np = (np.arange(128)[:, None] % 64 == np.arange(64)[None, :])
    idup_dram = nc.inline_tensor(idup_np.astype(bf), "idup").ap()
    # col 0 sums partitions 0-63, col 64 sums partitions 64-127 (keeps the
    # per-head sumsq rows at 32-aligned partitions 0 and 64)
    ind2_np = np.zeros((128, 128))
    ind2_np[:64, 0] = 1.0
    ind2_np[64:, 64] = 1.0
    ind2_dram = nc.inline_tensor(ind2_np.astype(bf), "ind2").ap()

    Exp = mybir.ActivationFunctionType.Exp
    Ln = mybir.ActivationFunctionType.Ln

    with tile.TileContext(nc) as tc, \
            tc.tile_pool(name="big", bufs=1) as big, \
            tc.tile_pool(name="cst", bufs=1) as cst, \
            tc.tile_pool(name="ptx", bufs=12) as ptxp, \
            tc.tile_pool(name="ptb", bufs=8) as ptbp, \
            tc.tile_pool(name="bia", bufs=3) as biap, \
            tc.tile_pool(name="sq", bufs=4) as sqp, \
            tc.tile_pool(name="small", bufs=2) as smp, \
            tc.tile_pool(name="osb", bufs=3) as osbp, \
            tc.tile_pool(name="drs", bufs=4, space="DRAM") as drsp, \
            tc.tile_pool(name="psA", bufs=3, space="PSUM") as psA, \
            tc.tile_pool(name="psB", bufs=2, space="PSUM") as psB:

        def bcast64(dst, src_row, tag):
            """broadcast a [1, n] SBUF row to [64, n] partitions of dst via a
            DRAM bounce (SBUF DMA sources cannot have stride-0 partitions)."""
            n = src_row.shape[-1]
            dt = drsp.tile([1, n], f32, tag=tag)
            nc.scalar.dma_start(out=dt[:], in_=src_row)
            nc.scalar.dma_start(out=dst, in_=dt[0:1, :].to_broadcast((64, n)))

        # ---- phase A: load everything (kv-path inputs first) ----
        wkv_sb = big.tile([128, 8, 2 * DH], bf16, tag="wkv")
        nc.sync.dma_start(wkv_sb[:], wkv.rearrange("(kt p) f -> p kt f", p=128))
        ctxT_sb = big.tile([128, 8, P], bf16, tag="ctxT")
        nc.sync.dma_start(ctxT_sb[:], ctxT.rearrange("(kt p) n -> p kt n", p=128))
        xT_sb = big.tile([128, 8, N], bf16, tag="xT")
        nc.sync.dma_start(xT_sb[:], xT.rearrange("(kt p) n -> p kt n", p=128))
        wc_sb = big.tile([128, 8, FL], bf16, tag="wc")
        nc.sync.dma_start(wc_sb[:], wc.rearrange("(kt p) f -> p kt f", p=128))
        wo_sb = big.tile([128, 4, DIM], bf16, tag="wo")
        nc.gpsimd.dma_start(wo_sb[:], wo.rearrange("(ft p) e -> p ft e", p=128))
        sdk_sb = cst.tile([DH, 1], f32, tag="sdk")
        nc.gpsimd.dma_start(sdk_sb[:], sdk)
        bandm_sb = cst.tile([128, BW], bf16, tag="bandm")
        nc.gpsimd.dma_start(bandm_sb[:], bandm_dram)
        idup_sb = cst.tile([128, 64], bf16, tag="idup")
        nc.gpsimd.dma_start(idup_sb[:], idup_dram)
        ind2_sb = cst.tile([128, 128], bf16, tag="ind2")
        nc.gpsimd.dma_start(ind2_sb[:], ind2_dram)
        eps_sb = cst.tile([128, 1], f32, tag="eps")
        nc.vector.memset(eps_sb[:], 1e-24)

        kvT_sb = big.tile([128, J], bf16, tag="kvT")      # [2d, j] raw kv
        kn_sb = big.tile([128, J], bf16, tag="kn")        # normalized k, dup'd
        va_sb = big.tile([128, 16, DH + 1], bf16, tag="va")  # v_aug, j-major
        qn_sb = big.tile([128, 4, N], bf16, tag="qn")     # normalized q
        att_sb = big.tile([128, 4, N], bf16, tag="att")   # avT/denom (features-major)

        # ---- phase B: kv projection, k-normalize, v transpose ----
        for jh in range(2):  # j halves of 1024 (0: prefix, 1: x)
            src = ctxT_sb if jh == 0 else xT_sb
            ps = psA.tile([128, 1024], f32, tag="A")
            for half in range(2):
                for kt in range(8):
                    nc.tensor.matmul(
                        ps[:, half * 512:(half + 1) * 512],
                        lhsT=wkv_sb[:, kt, :],
                        rhs=src[:, kt, half * 512:(half + 1) * 512],
                        start=(kt == 0), stop=(kt == 7))
            nc.vector.tensor_copy(out=kvT_sb[:, jh * 1024:(jh + 1) * 1024], in_=ps[:])
            for half in range(2):
                js = slice(jh * 1024 + half * 512, jh * 1024 + half * 512 + 512)
                pss = ps[:, half * 512:(half + 1) * 512]
                sq = sqp.tile([128, 512], bf16, tag="sq")
                nc.scalar.activation(sq[0:64, :], pss[0:64, :],
                                     mybir.ActivationFunctionType.Square)
                ssq = psB.tile([128, 512], f32, tag="B")
                nc.tensor.matmul(ssq[0:1, :], lhsT=ind2_sb[0:64, 0:1],
                                 rhs=sq[0:64, :], start=True, stop=True)
                rk = smp.tile([2, 512], f32, tag="rk")
                nc.scalar.activation(rk[0:1, :], ssq[0:1, :], Ln,
                                     bias=eps_sb[0:1])
                rkr = smp.tile([2, 512], f32, tag="rkr")
                nc.scalar.activation(rkr[0:1, :], rk[0:1, :], Exp, scale=-0.5)
                rkb = smp.tile([64, 512], f32, tag="rkb")
                bcast64(rkb[:], rkr[0:1, :], "drk")
                nc.vector.tensor_mul(kn_sb[0:64, js], kvT_sb[0:64, js], rkb[:])
                nc.vector.tensor_scalar_mul(kn_sb[0:64, js], kn_sb[0:64, js],
                                            sdk_sb[:])
        # duplicate kn into partitions 64-127 (for odd-parity heads)
        nc.gpsimd.dma_start(out=kn_sb[64:128, :], in_=kn_sb[0:64, :])
        # v transposes to j-major, build v_aug
        nc.vector.memset(va_sb[:, :, DH:DH + 1], 1.0)
        vt = psB.tile([128, 1024], bf16, tag="B")
        for jt in range(16):
            nc.tensor.transpose(
                vt[:, jt * 64:(jt + 1) * 64],
                kvT_sb[64:128, jt * 128:(jt + 1) * 128],
                idup_sb[64:128, :])
        nc.vector.tensor_copy(out=va_sb[:, :, 0:DH],
                              in_=vt[:].rearrange("p (t d) -> p t d", d=64))

        # ---- phase C: q projection + l2norm ----
        for ft in range(4):
            ps = psA.tile([128, 1024], f32, tag="A")
            for half in range(2):
                for kt in range(8):
                    nc.tensor.matmul(
                        ps[:, half * 512:(half + 1) * 512],
                        lhsT=wc_sb[:, kt, ft * 128:(ft + 1) * 128],
                        rhs=xT_sb[:, kt, half * 512:(half + 1) * 512],
                        start=(kt == 0), stop=(kt == 7))
            for half in range(2):
                qs = slice(half * 512, (half + 1) * 512)
                pss = ps[:, qs]
                sq = sqp.tile([128, 512], bf16, tag="sq")
                nc.scalar.activation(sq[:], pss, mybir.ActivationFunctionType.Square)
                ssq = psB.tile([128, 512], f32, tag="B")
                nc.tensor.matmul(ssq[:], lhsT=ind2_sb[:], rhs=sq[:],
                                 start=True, stop=True)
                rq = smp.tile([128, 512], f32, tag="rk")
                nc.scalar.activation(rq[:], ssq[:], Ln, bias=eps_sb[:])
                rqr = smp.tile([128, 512], f32, tag="rkr")
                nc.scalar.activation(rqr[:], rq[:], Exp, scale=-0.5)
                rqb = smp.tile([128, 512], f32, tag="rqb")
                bcast64(rqb[0:64, :], rqr[0:1, :], "drq0")
                bcast64(rqb[64:128, :], rqr[64:65, :], "drq1")
                nc.vector.tensor_mul(qn_sb[:, ft, qs], pss, rqb[:])

        # ---- phase D0: prefix band logits for all heads ----
        # Heads are processed in even/odd pairs: the even head lives on
        # partitions 0-63 and the odd head on 64-127 (via kn_dup), so
        # interleaved matmul emission runs on disjoint PE row groups.
        def head_view(h):
            base = (h % 2) * 64
            return (kn_sb[base:base + 64, :], qn_sb[base:base + 64, h // 2, :])

        ptbs = []
        for h in range(HL):
            ptbs.append(ptbp.tile([128, 8, BW], bf16, tag="ptb",
                                  name=f"ptb{h}"))
        for hp in range(4):
            for grp in range(2):
                bpss = [psA.tile([128, 1024], f32, tag="A", name=f"bps{hp}_{grp}_{k}")
                        for k in range(2)]
                for i in range(4):
                    ct = grp * 4 + i
                    c0 = 128 * ct
                    qw = min(BW, N - c0)
                    for pr in range(2):
                        kh, qh = head_view(2 * hp + pr)
                        nc.tensor.matmul(
                            bpss[pr][:, i * 256:i * 256 + qw],
                            lhsT=kh[:, c0:c0 + 128],
                            rhs=qh[:, c0:c0 + qw],
                            start=True, stop=True)
                for pr in range(2):
                    ptb = ptbs[2 * hp + pr]
                    bview = bpss[pr][:].rearrange(
                        "p (i x) -> p i x", x=256)[:, :, 0:BW]
                    nc.scalar.activation(ptb[:, grp * 4:(grp + 1) * 4, :],
                                         bview, Exp)
                    nc.vector.tensor_mul(
                        ptb[:, grp * 4:(grp + 1) * 4, :],
                        ptb[:, grp * 4:(grp + 1) * 4, :],
                        bandm_sb[:, None, :].to_broadcast((128, 4, BW)))

        # ---- phase D1/D2 + E: per query-chunk attention, then out-proj ----
        def emit_sims(h, qc):
            """sim matmuls + exp + bias-mult for one head/chunk; returns PT."""
            Q0 = qc * 512
            nct = 4 * (qc + 1)
            kh, qh = head_view(h)
            # one batched bias fetch for all col-tiles of this chunk; masked
            # regions are exactly 0 there, which also zeroes PT garbage
            bt = biap.tile([128, 8, 512], bf16, tag="bias")
            nc.gpsimd.dma_start(
                out=bt[:, 0:nct, :],
                in_=biasT[h].rearrange("(ct p) i -> p ct i", p=128)[
                    :, 0:nct, Q0:Q0 + 512])
            ptxs = []
            for pg in range((nct + 1) // 2):
                sps = psA.tile([128, 1024], f32, tag="A")
                ptx = ptxp.tile([128, 1024], bf16, tag="ptx")
                for i in range(2):
                    ct = pg * 2 + i
                    if ct >= nct:
                        continue
                    c0 = 128 * ct
                    off = max(0, c0 - Q0)
                    nc.tensor.matmul(
                        sps[:, i * 512 + off:(i + 1) * 512],
                        lhsT=kh[:, P + c0:P + c0 + 128],
                        rhs=qh[:, Q0 + off:Q0 + 512],
                        start=True, stop=True)
                nc.scalar.activation(ptx[:], sps[:], Exp)
                nc.vector.tensor_mul(
                    ptx[:],
                    ptx[:],
                    bt[:, pg * 2:pg * 2 + 2, :].rearrange("p a b -> p (a b)"))
                ptxs.append(ptx)
            return ptxs

        def emit_av(h, qc, ptxs):
            """AV accumulation + softmax normalize into att_sb."""
            Q0 = qc * 512
            nct = 4 * (qc + 1)
            base = (h % 2) * 64
            ft = h // 2
            ptb = ptbs[h]
            aps = psB.tile([128, 512], f32, tag="B")
            av_mms = []
            for ct in range(nct):
                c0 = 128 * ct
                off = max(0, c0 - Q0)
                av_mms.append((
                    aps[0:DH + 1, off:512], va_sb[:, 8 + ct, :],
                    ptxs[ct // 2][:, (ct % 2) * 512 + off:(ct % 2 + 1) * 512]))
            for ct in range(8):
                c0 = 128 * ct
                qw = min(BW, N - c0)
                lo = max(c0, Q0)
                hi = min(c0 + qw, Q0 + 512)
                if lo >= hi:
                    continue
                av_mms.append((
                    aps[0:DH + 1, lo - Q0:hi - Q0], va_sb[:, ct, :],
                    ptb[:, ct, lo - c0:hi - c0]))
            for i, (o, l, r) in enumerate(av_mms):
                nc.tensor.matmul(o, lhsT=l, rhs=r, start=(i == 0),
                                 stop=(i == len(av_mms) - 1))
            # normalize: 1/denom = exp(-ln(denom)), broadcast over rows
            rel = smp.tile([1, 512], f32, tag="rel")
            nc.scalar.activation(rel[:], aps[DH:DH + 1, :], Ln)
            rec = smp.tile([1, 512], f32, tag="rec")
            nc.scalar.activation(rec[:], rel[:], Exp, scale=-1.0)
            recb = smp.tile([64, 512], f32, tag="recb")
            bcast64(recb[:], rec[:], "drec")
            nc.vector.tensor_mul(
                att_sb[base:base + 64, ft, Q0:Q0 + 512],
                aps[0:64, :],
                recb[:])

        for qc in range(2):
            # software-pipeline by one head-pair: sims(hp) then avs(hp-1)
            pts = {}
            for hp in range(4):
                for pr in range(2):
                    pts[2 * hp + pr] = emit_sims(2 * hp + pr, qc)
                if hp > 0:
                    for pr in range(2):
                        emit_av(2 * (hp - 1) + pr, qc, pts.pop(2 * (hp - 1) + pr))
            for pr in range(2):
                emit_av(6 + pr, qc, pts.pop(6 + pr))

            # out-proj for this chunk
            for et in range(8):
                ops = psB.tile([128, 512], f32, tag="B")
                for ftile in range(4):
                    nc.tensor.matmul(
                        ops[:],
                        lhsT=wo_sb[:, ftile, et * 128:(et + 1) * 128],
                        rhs=att_sb[:, ftile, qc * 512:(qc + 1) * 512],
                        start=(ftile == 0), stop=(ftile == 3))
                o_sb = osbp.tile([128, 512], f32, tag="osb")
                nc.vector.tensor_copy(out=o_sb[:], in_=ops[:])
                nc.gpsimd.dma_start(
                    out=outT[et * 128:(et + 1) * 128, qc * 512:(qc + 1) * 512],
                    in_=o_sb[:])

    return nc


_NC = None


def _get_nc():
    global _NC
    if _NC is None:
        _NC = _build_nc()
    return _NC


def _prep_in_maps(x, prefix_context, attn_bias, gamma, Wq, Wkv, q_scale,
                  k_scale, Wo, mask):
    import ml_dtypes
    bf = ml_dtypes.bfloat16

    x = np.asarray(x, np.float32)
    prefix_context = np.asarray(prefix_context, np.float32)
    attn_bias = np.asarray(attn_bias, np.float32)
    gamma = np.asarray(gamma, np.float32)
    Wq = np.asarray(Wq, np.float32)
    Wkv = np.asarray(Wkv, np.float32)
    q_scale = np.asarray(q_scale, np.float32)
    k_scale = np.asarray(k_scale, np.float32)
    Wo = np.asarray(Wo, np.float32)
    mask = np.asarray(mask)

    tril = np.triu(np.ones((N, N), np.float32))  # keep key c <= query i ([c, i])
    sdk_np = (8.0 * q_scale * k_scale).astype(np.float32).reshape(DH, 1)
    wkvT = np.ascontiguousarray(Wkv.T).astype(bf)

    in_maps = []
    for c in CORES:
        b, g = c // 2, c % 2
        hs = slice(g * HL, (g + 1) * HL)
        # exp(bias) with causal kill and key-mask folded in, [h, c, i] layout
        eb = np.exp(attn_bias[hs].transpose(0, 2, 1)) * tril[None]
        maskf = mask[b].astype(np.float32)
        if not maskf.all():
            eb = eb * maskf[None, :, None]
        Wg = Wq[g * FL:(g + 1) * FL] * gamma[None, :]
        s = Wg.sum(axis=1)
        wcT = Wg.T - s[None, :] / DIM
        wog = Wo[:, g * FL:(g + 1) * FL]
        in_maps.append(dict(
            xT=np.ascontiguousarray(x[b].T).astype(bf),
            ctxT=np.ascontiguousarray(prefix_context[b].T).astype(bf),
            biasT=np.ascontiguousarray(eb).astype(bf),
            wc=np.ascontiguousarray(wcT).astype(bf),
            wkv=wkvT,
            wo=np.ascontiguousarray(wog.T).astype(bf),
            sdk=sdk_np,
        ))
    return in_maps


def kernel(**inputs):
    from concourse.bass_utils import run_bass_kernel_spmd

    nc = _get_nc()
    in_maps = _prep_in_maps(**inputs)
    res = run_bass_kernel_spmd(nc, in_maps, CORES).results
    out = np.empty((B, N, DIM), np.float32)
    for b in range(B):
        out[b] = (np.asarray(res[2 * b]["outT"], np.float32)
                  + np.asarray(res[2 * b + 1]["outT"], np.float32)).T
    return out


# revision 39
# speedup vs baseline: 1.1271x; 1.1271x over previous
# Distributed sparse-attention kernel for Trainium2 (8 NeuronCores).
#
# Sharding: core c = (batch b = c//2, head-group g = c%2 of 8 heads).
# Each core computes, for its (b, g):
#   q  = meancenter(x) @ Wc          (LN rstd cancels under l2norm; gamma and
#                                     mean-centering folded into Wc on host)
#   kv = [prefix; x] @ Wkv.T         (MQA single head, replicated per batch)
#   qn = l2norm(q) ; kn = l2norm(k) * (8 * q_scale * k_scale)
#   logits[c_key, i_query] = kn.T qn  (computed transposed, only the causal
#                                     triangle over x-cols and a 16-wide band
#                                     over prefix-cols)
#   P = exp(logits) * exp_bias       (bias/causal/key-mask folded into a
#                                     multiplicative exp(bias) table on host)
#   avT[d, i] = sum_j P[j, i] v[j, d], denom via an appended ones column of v
#   outT_partial = Wo_g.T @ (avT / denom)
# Host sums the two head-group partials per batch and transposes back.

import numpy as np

B, N, P, DIM, HEADS, DH = 4, 1024, 1024, 1024, 16, 64
HL = 8                 # heads per core
FL = HL * DH           # 512 local q features
J = P + N              # 2048 keys
WIND = 16              # prefix cond-window
BW = 144               # band tile width (128 cols + 16 window - 1, padded)
NEG = -1e30
CORES = list(range(8))


def _build_band_mask():
    # band[r, t] = 1 if key col (c0+r) is attended by query (c0+t):  0 <= t-r < WIND
    r = np.arange(128)[:, None]
    t = np.arange(BW)[None, :]
    return ((t - r >= 0) & (t - r < WIND)).astype(np.float32)


def _patch_tile_drain():
    """walrus in this image only encodes ~2 sem waits on a CTRL (Drain/Nop)
    instruction; Tile's exit drain attaches every outstanding sem wait to a
    single drain.  Split the waits across extra sync-engine nops."""
    import concourse.tile as tile_mod
    from concourse import mybir
    from concourse.vector_clock import ScopedClock

    if getattr(tile_mod.TileContext, "_drain_split_patch", False):
        return
    MAXW = 1

    _ENGS = {
        mybir.EngineType.PE, mybir.EngineType.Activation,
        mybir.EngineType.Pool, mybir.EngineType.DVE, mybir.EngineType.SP,
    }
    _LIMITS = {}
    _nsplit = [0]
    orig_add = tile_mod.TileContext._add_instruction

    def _add_instruction(self, inst):
        si = inst.sync_info
        lim = _LIMITS.get(inst.engine, 1)
        if (si is not None and si.on_wait and len(si.on_wait) > lim
                and inst.engine in _ENGS):
            waits = list(si.on_wait)
            keep = waits[:lim]
            rest = waits[lim:]
            inst.sync_info = mybir.SyncInfo(
                on_wait=keep, on_update=list(si.on_update or []))
            for i in range(0, len(rest), MAXW):
                _nsplit[0] += 1
                nop = mybir.InstNoOp(
                    name=f"{inst.name}-ws{_nsplit[0]}", ins=[], outs=[])
                nop.engine = inst.engine
                nop.sync_info = mybir.SyncInfo(
                    on_wait=rest[i:i + MAXW], on_update=[])
                orig_add(self, nop)
        orig_add(self, inst)

    tile_mod.TileContext._add_instruction = _add_instruction

    def _drain_and_barrier(self, tick_clock, wait_clock):
        drain_inst = self.nc.sync.drain()
        wait_clock.add_sem_waits(
            drain_inst.ins, ScopedClock({None: tick_clock.global_clock})
        )
        si = drain_inst.ins.sync_info
        waits = list(si.on_wait or []) if si is not None else []
        if len(waits) > MAXW:
            ups = list(si.on_update or []) if si is not None else []
            drain_inst.ins.sync_info = mybir.SyncInfo(on_wait=[], on_update=ups)
            for i in range(0, len(waits), MAXW):
                nop = self.nc.sync.nop(nofuse=True)
                nop.ins.sync_info = mybir.SyncInfo(
                    on_wait=waits[i:i + MAXW], on_update=[])
        self.nc.all_engine_barrier()
        assert self.sems is not None
        popped = self.nc._tile_sem_poison_stack.pop()
        assert popped is self._sem_poison
        self.nc.clear_and_free_semaphores(list(self.sems.allocated().values()))
        self.nc.all_engine_barrier()

    tile_mod.TileContext._drain_and_barrier = _drain_and_barrier
    tile_mod.TileContext._drain_split_patch = True


def _build_nc():
    import ml_dtypes
    import concourse.bass as bass
    import concourse.tile as tile
    from concourse import mybir

    _patch_tile_drain()

    f32 = mybir.dt.float32
    bf16 = mybir.dt.bfloat16
    bf = ml_dtypes.bfloat16

    nc = bass.Bass("TRN2", target_bir_lowering=False, debug=False)

    xT = nc.dram_tensor("xT", [DIM, N], bf16, kind="ExternalInput").ap()
    ctxT = nc.dram_tensor("ctxT", [DIM, P], bf16, kind="ExternalInput").ap()
    biasT = nc.dram_tensor("biasT", [HL, N, N], bf16, kind="ExternalInput").ap()
    wc = nc.dram_tensor("wc", [DIM, FL], bf16, kind="ExternalInput").ap()
    wkv = nc.dram_tensor("wkv", [DIM, 2 * DH], bf16, kind="ExternalInput").ap()
    wo = nc.dram_tensor("wo", [FL, DIM], bf16, kind="ExternalInput").ap()
    sdk = nc.dram_tensor("sdk", [DH, 1], f32, kind="ExternalInput").ap()
    outT = nc.dram_tensor("outT", [DIM, N], f32, kind="ExternalOutput").ap()

    bandm_dram = nc.inline_tensor(_build_band_mask().astype(bf), "bandm").ap()
    idup_np = (np.arange(128)[:, None] % 64 == np.arange(64)[None, :])
    idup_dram = nc.inline_tensor(idup_np.astype(bf), "idup").ap()
    # col 0 sums partitions 0-63, col 64 sums partitions 64-127 (keeps the
    # per-head sumsq rows at 32-aligned partitions 0 and 64)
    ind2_np = np.zeros((128, 128))
    ind2_np[:64, 0] = 1.0
    ind2_np[64:, 64] = 1.0
    ind2_dram = nc.inline_tensor(ind2_np.astype(bf), "ind2").ap()

    Exp = mybir.ActivationFunctionType.Exp
    Ln = mybir.ActivationFunctionType.Ln

    with tile.TileContext(nc) as tc, \
            tc.tile_pool(name="big", bufs=1) as big, \
            tc.tile_pool(name="cst", bufs=1) as cst, \
            tc.tile_pool(name="ptx", bufs=12) as ptxp, \
            tc.tile_pool(name="ptb", bufs=8) as ptbp, \
            tc.tile_pool(name="bia", bufs=3) as biap, \
            tc.tile_pool(name="sq", bufs=4) as sqp, \
            tc.tile_pool(name="small", bufs=2) as smp, \
            tc.tile_pool(name="osb", bufs=3) as osbp, \
            tc.tile_pool(name="drs", bufs=4, space="DRAM") as drsp, \
            tc.tile_pool(name="psA", bufs=3, space="PSUM") as psA, \
            tc.tile_pool(name="psB", bufs=2, space="PSUM") as psB:

        def bcast64(dst, src_row, tag):
            """broadcast a [1, n] SBUF row to [64, n] partitions of dst via a
            DRAM bounce (SBUF DMA sources cannot have stride-0 partitions)."""
            n = src_row.shape[-1]
            dt = drsp.tile([1, n], f32, tag=tag)
            nc.scalar.dma_start(out=dt[:], in_=src_row)
            nc.scalar.dma_start(out=dst, in_=dt[0:1, :].to_broadcast((64, n)))

        # ---- phase A: load everything (kv-path inputs first, loads split
        # across chunks + engines so they land on many DMA queues) ----
        wkv_sb = big.tile([128, 8, 2 * DH], bf16, tag="wkv")
        nc.sync.dma_start(wkv_sb[:], wkv.rearrange("(kt p) f -> p kt f", p=128))
        ctxT_sb = big.tile([128, 8, P], bf16, tag="ctxT")
        ctxr = ctxT.rearrange("(kt p) n -> p kt n", p=128)
        xT_sb = big.tile([128, 8, N], bf16, tag="xT")
        xr = xT.rearrange("(kt p) n -> p kt n", p=128)
        for k in range(4):
            eng = (nc.sync, nc.gpsimd, nc.scalar, nc.sync)[k % 4]
            eng.dma_start(ctxT_sb[:, 2 * k:2 * k + 2, :], ctxr[:, 2 * k:2 * k + 2, :])
        for k in range(4):
            eng = (nc.gpsimd, nc.scalar, nc.sync, nc.gpsimd)[k % 4]
            eng.dma_start(xT_sb[:, 2 * k:2 * k + 2, :], xr[:, 2 * k:2 * k + 2, :])
        wc_sb = big.tile([128, 8, FL], bf16, tag="wc")
        wcr = wc.rearrange("(kt p) f -> p kt f", p=128)
        for k in range(2):
            (nc.scalar, nc.sync)[k].dma_start(
                wc_sb[:, 4 * k:4 * k + 4, :], wcr[:, 4 * k:4 * k + 4, :])
        wo_sb = big.tile([128, 4, DIM], bf16, tag="wo")
        nc.gpsimd.dma_start(wo_sb[:], wo.rearrange("(ft p) e -> p ft e", p=128))
        sdk_sb = cst.tile([DH, 1], f32, tag="sdk")
        nc.gpsimd.dma_start(sdk_sb[:], sdk)
        bandm_sb = cst.tile([128, BW], bf16, tag="bandm")
        nc.gpsimd.dma_start(bandm_sb[:], bandm_dram)
        idup_sb = cst.tile([128, 64], bf16, tag="idup")
        nc.gpsimd.dma_start(idup_sb[:], idup_dram)
        ind2_sb = cst.tile([128, 128], bf16, tag="ind2")
        nc.gpsimd.dma_start(ind2_sb[:], ind2_dram)
        eps_sb = cst.tile([128, 1], f32, tag="eps")
        nc.vector.memset(eps_sb[:], 1e-24)

        kvT_sb = big.tile([128, J], bf16, tag="kvT")      # [2d, j] raw kv
        kn_sb = big.tile([128, J], bf16, tag="kn")        # normalized k, dup'd
        va_sb = big.tile([128, 16, DH + 1], bf16, tag="va")  # v_aug, j-major
        qn_sb = big.tile([128, 4, N], bf16, tag="qn")     # normalized q
        att_sb = big.tile([128, 4, N], bf16, tag="att")   # avT/denom (features-major)

        # ---- phase B: kv projection, k-normalize, v transpose ----
        for jh in range(2):  # j halves of 1024 (0: prefix, 1: x)
            src = ctxT_sb if jh == 0 else xT_sb
            ps = psA.tile([128, 1024], f32, tag="A")
            for half in range(2):
                for kt in range(8):
                    nc.tensor.matmul(
                        ps[:, half * 512:(half + 1) * 512],
                        lhsT=wkv_sb[:, kt, :],
                        rhs=src[:, kt, half * 512:(half + 1) * 512],
                        start=(kt == 0), stop=(kt == 7))
            nc.vector.tensor_copy(out=kvT_sb[:, jh * 1024:(jh + 1) * 1024], in_=ps[:])
            for half in range(2):
                js = slice(jh * 1024 + half * 512, jh * 1024 + half * 512 + 512)
                pss = ps[:, half * 512:(half + 1) * 512]
                sq = sqp.tile([128, 512], bf16, tag="sq")
                nc.scalar.activation(sq[0:64, :], pss[0:64, :],
                                     mybir.ActivationFunctionType.Square)
                ssq = psB.tile([128, 512], f32, tag="B")
                nc.tensor.matmul(ssq[0:1, :], lhsT=ind2_sb[0:64, 0:1],
                                 rhs=sq[0:64, :], start=True, stop=True)
                rk = smp.tile([2, 512], f32, tag="rk")
                nc.scalar.activation(rk[0:1, :], ssq[0:1, :], Ln,
                                     bias=eps_sb[0:1])
                rkr = smp.tile([2, 512], f32, tag="rkr")
                nc.scalar.activation(rkr[0:1, :], rk[0:1, :], Exp, scale=-0.5)
                rkb = smp.tile([64, 512], f32, tag="rkb")
                bcast64(rkb[:], rkr[0:1, :], "drk")
                nc.vector.tensor_mul(kn_sb[0:64, js], kvT_sb[0:64, js], rkb[:])
                nc.vector.tensor_scalar_mul(kn_sb[0:64, js], kn_sb[0:64, js],
                                            sdk_sb[:])
        # duplicate kn into partitions 64-127 (for odd-parity heads)
        nc.gpsimd.dma_start(out=kn_sb[64:128, :], in_=kn_sb[0:64, :])
        # v transposes to j-major, build v_aug
        nc.vector.memset(va_sb[:, :, DH:DH + 1], 1.0)
        vt = psB.tile([128, 1024], bf16, tag="B")
        for jt in range(16):
            nc.tensor.transpose(
                vt[:, jt * 64:(jt + 1) * 64],
                kvT_sb[64:128, jt * 128:(jt + 1) * 128],
                idup_sb[64:128, :])
        nc.vector.tensor_copy(out=va_sb[:, :, 0:DH],
                              in_=vt[:].rearrange("p (t d) -> p t d", d=64))

        # ---- phase C: q projection + l2norm ----
        for ft in range(4):
            ps = psA.tile([128, 1024], f32, tag="A")
            for half in range(2):
                for kt in range(8):
                    nc.tensor.matmul(
                        ps[:, half * 512:(half + 1) * 512],
                        lhsT=wc_sb[:, kt, ft * 128:(ft + 1) * 128],
                        rhs=xT_sb[:, kt, half * 512:(half + 1) * 512],
                        start=(kt == 0), stop=(kt == 7))
            for half in range(2):
                qs = slice(half * 512, (half + 1) * 512)
                pss = ps[:, qs]
                sq = sqp.tile([128, 512], bf16, tag="sq")
                nc.scalar.activation(sq[:], pss, mybir.ActivationFunctionType.Square)
                ssq = psB.tile([128, 512], f32, tag="B")
                nc.tensor.matmul(ssq[:], lhsT=ind2_sb[:], rhs=sq[:],
                                 start=True, stop=True)
                rq = smp.tile([128, 512], f32, tag="rk")
                nc.scalar.activation(rq[:], ssq[:], Ln, bias=eps_sb[:])
                rqr = smp.tile([128, 512], f32, tag="rkr")
                nc.scalar.activation(rqr[:], rq[:], Exp, scale=-0.5)
                rqb = smp.tile([128, 512], f32, tag="rqb")
                bcast64(rqb[0:64, :], rqr[0:1, :], "drq0")
                bcast64(rqb[64:128, :], rqr[64:65, :], "drq1")
                nc.vector.tensor_mul(qn_sb[:, ft, qs], pss, rqb[:])

        # ---- phase D0: prefix band logits for all heads ----
        # Heads are processed in even/odd pairs: the even head lives on
        # partitions 0-63 and the odd head on 64-127 (via kn_dup), so
        # interleaved matmul emission runs on disjoint PE row groups.
        def head_view(h):
            base = (h % 2) * 64
            return (kn_sb[base:base + 64, :], qn_sb[base:base + 64, h // 2, :])

        ptbs = []
        for h in range(HL):
            ptbs.append(ptbp.tile([128, 8, BW], bf16, tag="ptb",
                                  name=f"ptb{h}"))
        for hp in range(4):
            for grp in range(2):
                bpss = [psA.tile([128, 1024], f32, tag="A", name=f"bps{hp}_{grp}_{k}")
                        for k in range(2)]
                for i in range(4):
                    ct = grp * 4 + i
                    c0 = 128 * ct
                    qw = min(BW, N - c0)
                    for pr in range(2):
                        kh, qh = head_view(2 * hp + pr)
                        nc.tensor.matmul(
                            bpss[pr][:, i * 256:i * 256 + qw],
                            lhsT=kh[:, c0:c0 + 128],
                            rhs=qh[:, c0:c0 + qw],
                            start=True, stop=True)
                for pr in range(2):
                    ptb = ptbs[2 * hp + pr]
                    bview = bpss[pr][:].rearrange(
                        "p (i x) -> p i x", x=256)[:, :, 0:BW]
                    nc.scalar.activation(ptb[:, grp * 4:(grp + 1) * 4, :],
                                         bview, Exp)
                    nc.vector.tensor_mul(
                        ptb[:, grp * 4:(grp + 1) * 4, :],
                        ptb[:, grp * 4:(grp + 1) * 4, :],
                        bandm_sb[:, None, :].to_broadcast((128, 4, BW)))

        # ---- phase D1/D2 + E: per query-chunk attention, then out-proj ----
        def emit_sims(h, qc):
            """sim matmuls + exp + bias-mult for one head/chunk; returns PT."""
            Q0 = qc * 512
            nct = 4 * (qc + 1)
            kh, qh = head_view(h)
            # bias fetch for all col-tiles of this chunk, split in 2-ct DMAs
            # so transfers spread over several queues; masked regions are
            # exactly 0, which also zeroes PT garbage
            bt = biap.tile([128, 8, 512], bf16, tag="bias")
            btr = biasT[h].rearrange("(ct p) i -> p ct i", p=128)
            for pg in range((nct + 1) // 2):
                nc.gpsimd.dma_start(
                    out=bt[:, 2 * pg:2 * pg + 2, :],
                    in_=btr[:, 2 * pg:2 * pg + 2, Q0:Q0 + 512])
            ptxs = []
            for pg in range((nct + 1) // 2):
                sps = psA.tile([128, 1024], f32, tag="A")
                ptx = ptxp.tile([128, 1024], bf16, tag="ptx")
                for i in range(2):
                    ct = pg * 2 + i
                    if ct >= nct:
                        continue
                    c0 = 128 * ct
                    off = max(0, c0 - Q0)
                    nc.tensor.matmul(
                        sps[:, i * 512 + off:(i + 1) * 512],
                        lhsT=kh[:, P + c0:P + c0 + 128],
                        rhs=qh[:, Q0 + off:Q0 + 512],
                        start=True, stop=True)
                nc.scalar.activation(ptx[:], sps[:], Exp)
                nc.vector.tensor_mul(
                    ptx[:],
                    ptx[:],
                    bt[:, pg * 2:pg * 2 + 2, :].rearrange("p a b -> p (a b)"))
                ptxs.append(ptx)
            return ptxs

        def emit_av(h, qc, ptxs):
            """AV accumulation + softmax normalize into att_sb."""
            Q0 = qc * 512
            nct = 4 * (qc + 1)
            base = (h % 2) * 64
            ft = h // 2
            ptb = ptbs[h]
            aps = psB.tile([128, 512], f32, tag="B")
            av_mms = []
            for ct in range(nct):
                c0 = 128 * ct
                off = max(0, c0 - Q0)
                av_mms.append((
                    aps[0:DH + 1, off:512], va_sb[:, 8 + ct, :],
                    ptxs[ct // 2][:, (ct % 2) * 512 + off:(ct % 2 + 1) * 512]))
            for ct in range(8):
                c0 = 128 * ct
                qw = min(BW, N - c0)
                lo = max(c0, Q0)
                hi = min(c0 + qw, Q0 + 512)
                if lo >= hi:
                    continue
                av_mms.append((
                    aps[0:DH + 1, lo - Q0:hi - Q0], va_sb[:, ct, :],
                    ptb[:, ct, lo - c0:hi - c0]))
            for i, (o, l, r) in enumerate(av_mms):
                nc.tensor.matmul(o, lhsT=l, rhs=r, start=(i == 0),
                                 stop=(i == len(av_mms) - 1))
            # normalize: 1/denom = exp(-ln(denom)), broadcast over rows
            rel = smp.tile([1, 512], f32, tag="rel")
            nc.scalar.activation(rel[:], aps[DH:DH + 1, :], Ln)
            rec = smp.tile([1, 512], f32, tag="rec")
            nc.scalar.activation(rec[:], rel[:], Exp, scale=-1.0)
            recb = smp.tile([64, 512], f32, tag="recb")
            bcast64(recb[:], rec[:], "drec")
            nc.vector.tensor_mul(
                att_sb[base:base + 64, ft, Q0:Q0 + 512],
                aps[0:64, :],
                recb[:])

        for qc in range(2):
            # per head-pair: both heads' sims, then both heads' avs
            for hp in range(4):
                pts = [emit_sims(2 * hp + pr, qc) for pr in range(2)]
                for pr in range(2):
                    emit_av(2 * hp + pr, qc, pts[pr])

            # out-proj for this chunk
            for et in range(8):
                ops = psB.tile([128, 512], f32, tag="B")
                for ftile in range(4):
                    nc.tensor.matmul(
                        ops[:],
                        lhsT=wo_sb[:, ftile, et * 128:(et + 1) * 128],
                        rhs=att_sb[:, ftile, qc * 512:(qc + 1) * 512],
                        start=(ftile == 0), stop=(ftile == 3))
                o_sb = osbp.tile([128, 512], f32, tag="osb")
                nc.vector.tensor_copy(out=o_sb[:], in_=ops[:])
                nc.gpsimd.dma_start(
                    out=outT[et * 128:(et + 1) * 128, qc * 512:(qc + 1) * 512],
                    in_=o_sb[:])

    return nc


_NC = None


def _get_nc():
    global _NC
    if _NC is None:
        _NC = _build_nc()
    return _NC


def _prep_in_maps(x, prefix_context, attn_bias, gamma, Wq, Wkv, q_scale,
                  k_scale, Wo, mask):
    import ml_dtypes
    bf = ml_dtypes.bfloat16

    x = np.asarray(x, np.float32)
    prefix_context = np.asarray(prefix_context, np.float32)
    attn_bias = np.asarray(attn_bias, np.float32)
    gamma = np.asarray(gamma, np.float32)
    Wq = np.asarray(Wq, np.float32)
    Wkv = np.asarray(Wkv, np.float32)
    q_scale = np.asarray(q_scale, np.float32)
    k_scale = np.asarray(k_scale, np.float32)
    Wo = np.asarray(Wo, np.float32)
    mask = np.asarray(mask)

    tril = np.triu(np.ones((N, N), np.float32))  # keep key c <= query i ([c, i])
    sdk_np = (8.0 * q_scale * k_scale).astype(np.float32).reshape(DH, 1)
    wkvT = np.ascontiguousarray(Wkv.T).astype(bf)

    in_maps = []
    for c in CORES:
        b, g = c // 2, c % 2
        hs = slice(g * HL, (g + 1) * HL)
        # exp(bias) with causal kill and key-mask folded in, [h, c, i] layout
        eb = np.exp(attn_bias[hs].transpose(0, 2, 1)) * tril[None]
        maskf = mask[b].astype(np.float32)
        if not maskf.all():
            eb = eb * maskf[None, :, None]
        Wg = Wq[g * FL:(g + 1) * FL] * gamma[None, :]
        s = Wg.sum(axis=1)
        wcT = Wg.T - s[None, :] / DIM
        wog = Wo[:, g * FL:(g + 1) * FL]
        in_maps.append(dict(
            xT=np.ascontiguousarray(x[b].T).astype(bf),
            ctxT=np.ascontiguousarray(prefix_context[b].T).astype(bf),
            biasT=np.ascontiguousarray(eb).astype(bf),
            wc=np.ascontiguousarray(wcT).astype(bf),
            wkv=wkvT,
            wo=np.ascontiguousarray(wog.T).astype(bf),
            sdk=sdk_np,
        ))
    return in_maps


def kernel(**inputs):
    from concourse.bass_utils import run_bass_kernel_spmd

    nc = _get_nc()
    in_maps = _prep_in_maps(**inputs)
    res = run_bass_kernel_spmd(nc, in_maps, CORES).results
    out = np.empty((B, N, DIM), np.float32)
    for b in range(B):
        out[b] = (np.asarray(res[2 * b]["outT"], np.float32)
                  + np.asarray(res[2 * b + 1]["outT"], np.float32)).T
    return out
